# revision 27
# baseline (speedup 1.0000x reference)
"""BlockGRUCell Trainium2 kernel.

Computation (per reference):
  hx = concat([h, x], -1)                       # (B, 2048)
  gate[b, 192g+o] = sum_i hx[b, 128g+i] * W[g, o, i]   # block-diagonal matmul
  r, c, u = split(gate + bias, 3)               # bias == 0 from setup_inputs
  h_new = sigmoid(u) * tanh(sigmoid(r) * c) + (1 - sigmoid(u)) * h

Sharding: data-parallel over batch across 8 NeuronCores (2048 rows each),
weights replicated.

Design (evolved from the 95.7us baseline via trace analysis; ~70us now):
  - The baseline was DVE-bound (fp32 tensor_tensor is 1x ~= 1152ns per
    1024-wide op, 78us busy) with ACT (55us) and DMA (24MB -> 72us) close
    behind.  Fixes:
  - All elementwise tensors are bf16 -> DVE tensor_tensor runs 2x_1P
    (2 elem/cycle/lane).  Only the rc = sigmoid(r)*c multiply reads PSUM
    (f32, 1x) since TRN2 matmul can only write f32 PSUM.
  - h is loaded as bf16 (not f32) and the output is stored as bf16 and
    upcast on host: HBM traffic 24MB -> 16MB per core.
  - Tiles processed in pairs of 128 rows; tanh + blend run at free-dim
    2048 to amortize per-op overhead and semaphores.
  - ACTIVATEs are emitted with immediate bias (bass's float->AP bias
    conversion costs a measured ~92ns/op in const-AP reads).
  - Output stores go through the SWDGE (gpsimd) DMA ring so they never
    block the HWDGE (sync) load ring's FIFO dispatch; weight loads ride
    the second HWDGE ring (ACT-dispatched) in parallel with hxt loads.
  - Pair 0's panels arrive as 256KB pieces (sub-tile deps) so the first
    tile's matmuls start as early as possible during the pipeline fill.

Steady state is ACT/DVE co-bound at ~48-49us busy each (the 3 LUT
passes/element are irreducible: ACT is the only LUT engine at 1 elem/
cycle/lane; GpSimd has no PSUM access and its stock tensor ops deadlock
against DVE's shared-port lock, measured as NRT_EXEC_UNIT_UNRECOVERABLE).

Per core, per 128-row tile: 20 block matmuls (bf16, f32 PSUM, split at
512-col PSUM bank crossings) into gR/gC/gU [128,1024] panels; ACT does
sigmoid(gR)->reset, sigmoid(gU)->upd (bf16, into pair tiles); DVE does
rc = gC*reset (1x, PSUM).  Per pair: ACT tanh(rc_pair)->cand (2048 wide);
DVE: d = cand - h, e = upd*d, hn = h + e; store.
"""

import numpy as np
import ml_dtypes

import concourse.bass as bass
import concourse.bacc as bacc
import concourse.tile as tile
import concourse.mybir as mybir
from concourse.bass_utils import run_bass_kernel_spmd

N_CORES = 8
BATCH = 16384
BS = BATCH // N_CORES            # rows per core
P = 128
NT = BS // P                     # 128-row tiles per core
NP = NT // 2                     # tile pairs per core
HID = 1024
G = 16                           # feature blocks
IN_PER = 128
OUT_PER = 192
GATE = 3 * HID                   # 3072
PSUM_BANK_F32 = 512

F32 = mybir.dt.float32
BF16 = mybir.dt.bfloat16
AFT = mybir.ActivationFunctionType
MULT = mybir.AluOpType.mult

# dtype knobs. fp8e4 hxt was tried and reverted: the compiler never
# engaged fast-weight-load (LDWEIGHTS stayed ~128ns), the DMA savings were
# eaten by small-transfer inefficiency, and ACT/DVE op durations regressed
# ~1.2x in that build; bf16 also keeps 2x the error margin.
HXT_DT = BF16
WT_DT = BF16
WS = 1.0                         # weight pre-scale, unwound via ACT scale


def _act(nc, out, in_, func, scale=1.0):
    """ACTIVATE with immediate bias/scale/alpha.  bass's activation()
    lowers a float bias to a [P,1] const AP, whose per-op read costs
    ~92ns (HW-measured on 200-op chains); the ISA accepts immediates."""
    eng = nc.scalar
    ins = [eng.lower_ap(in_)]
    for val in (0.0, scale, 0.0):   # bias, scale, alpha
        ins.append(mybir.ImmediateValue(dtype=mybir.dt.float32, value=val))
    return eng.add_instruction(
        mybir.InstActivation(
            name=nc.get_next_instruction_name(),
            func=func,
            ins=ins,
            outs=[eng.lower_ap(out)],
        ))


def _body(tc, nc, hxt_d, h_d, wt_d, out_d):
    inv_ws = 1.0 / WS
    with (
        tc.tile_pool(name="consts", bufs=1) as consts,
        tc.tile_pool(name="io", bufs=3) as io,
        tc.tile_pool(name="panels", bufs=2) as panels,
        tc.tile_pool(name="gatep", bufs=4, space="PSUM") as gatep,
    ):
        # warm the sigmoid/tanh ACT table during the initial DMAs (the
        # ~2.7us ACT_TABLE_LOAD otherwise lands on tile 0's critical path)
        warm = consts.tile([P, 1], F32)
        nc.vector.memset(warm, 0.0)
        nc.scalar.activation(warm, warm, AFT.Sigmoid)

        # weight loads ride the second HWDGE ring (ACT engine dispatch, idle
        # at startup) so they stream in parallel with the hxt loads on the
        # sync ring, in r/c/u pieces so each gate region unblocks asap
        wt_s = consts.tile([P, GATE], WT_DT)
        for k in range(3):
            nc.scalar.dma_start(out=wt_s[:, k * HID:(k + 1) * HID],
                                in_=wt_d[:, k * HID:(k + 1) * HID])

        for q in range(NP):
            # one [128, 4096] tile per pair, h-blocks in the low half,
            # x-blocks in the high half; steady state is one contiguous 1MB
            # load, pair 0 arrives in three pieces (sub-tile deps let tile
            # 0's h-block matmuls start after the first 256KB)
            hxt = io.tile([P, 2 * G * P], HXT_DT, tag="hxt")
            if q == 0:
                nc.sync.dma_start(out=hxt[:, 0:HID], in_=hxt_d[0, :, 0:HID])
                nc.sync.dma_start(out=hxt[:, HID:2 * HID],
                                  in_=hxt_d[0, :, HID:2 * HID])
                nc.sync.dma_start(out=hxt[:, 2 * HID:],
                                  in_=hxt_d[0, :, 2 * HID:])
            else:
                nc.sync.dma_start(out=hxt, in_=hxt_d[q])
            h2 = io.tile([P, 2 * HID], BF16, tag="h2")
            if q > 0:
                nc.sync.dma_start(out=h2, in_=h_d[q])
            out2 = None
            if q >= NP - 2:
                out2 = io.tile([P, 2 * HID], BF16, tag="out2")

            reset2 = panels.tile([P, 2 * HID], BF16, tag="reset")
            upd2 = panels.tile([P, 2 * HID], BF16, tag="upd")
            rc2 = panels.tile([P, 2 * HID], BF16, tag="rc")
            cand2 = panels.tile([P, 2 * HID], BF16, tag="cand")
            dd = panels.tile([P, 2 * HID], BF16, tag="dd")
            ee = panels.tile([P, 2 * HID], BF16, tag="ee")

            for s in (0, 1):
                gR = gatep.tile([P, HID], F32, tag="gate")
                gC = gatep.tile([P, HID], F32, tag="gate")
                gU = gatep.tile([P, HID], F32, tag="gate")
                gs = (gR, gC, gU)

                for g in range(G):
                    base = 2 * HID * (g // 8) + HID * s + (g % 8) * P
                    lhsT = hxt[:, base:base + P]
                    w0 = g * OUT_PER
                    # split matmul writes at PSUM bank (512 f32) boundaries
                    c0 = w0
                    while c0 < w0 + OUT_PER:
                        c1 = min(w0 + OUT_PER,
                                 (c0 // PSUM_BANK_F32 + 1) * PSUM_BANK_F32)
                        gate = gs[c0 // HID]
                        nc.tensor.matmul(
                            gate[:, c0 % HID:(c0 % HID) + c1 - c0],
                            lhsT, wt_s[:, c0:c1], start=True, stop=True)
                        c0 = c1

                if q == 0 and s == 0:
                    # defer the first h load until after the critical-path
                    # hxt/wt DMAs are queued (h is only needed at the blend)
                    nc.sync.dma_start(out=h2, in_=h_d[0])

                sl = slice(s * HID, (s + 1) * HID)
                _act(nc, reset2[:, sl], gR, AFT.Sigmoid, inv_ws)
                _act(nc, upd2[:, sl], gU, AFT.Sigmoid, inv_ws)
                nc.vector.tensor_tensor(rc2[:, sl], gC, reset2[:, sl], MULT)

            # pair epilogue at free-dim 2048; the last pair's DVE chain runs
            # in 512-wide quarters (tanh in halves) so the serial ACT<->DVE
            # tail drains finer-grained and the final store streams out early
            if q < NP - 1:
                tanh_splits = [(0, 2 * HID)]
                dve_splits = [(0, 2 * HID)]
            else:
                tanh_splits = [(0, HID), (HID, 2 * HID)]
                dve_splits = [(0, HID), (HID, 2 * HID)]
            for a, b in tanh_splits:
                _act(nc, cand2[:, a:b], rc2[:, a:b], AFT.Tanh, inv_ws)
            for a, b in dve_splits:
                nc.vector.tensor_sub(dd[:, a:b], cand2[:, a:b], h2[:, a:b])
                nc.vector.tensor_mul(ee[:, a:b], upd2[:, a:b], dd[:, a:b])
                # hn = h + e: steady-state pairs fold the add into the SDMA
                # CCE (accumulate-DMA onto the h2 tile, then store it) —
                # that removes 8x1226ns from the DVE, which is co-bound with
                # ACT.  Stores ride the SWDGE (gpsimd) ring so they never
                # stall the HWDGE load ring's FIFO dispatch.  The last two
                # pairs keep the DVE add + sync-ring stores: the CCE hop
                # would lengthen the drain tail, and SWDGE's final drain is
                # slow.
                if q < NP - 2:
                    nc.gpsimd.dma_start(out=h2[:, a:b], in_=ee[:, a:b],
                                        accum_op=mybir.AluOpType.add)
                    nc.gpsimd.dma_start(out=out_d[q][:, a:b], in_=h2[:, a:b])
                else:
                    nc.vector.tensor_add(out2[:, a:b], h2[:, a:b], ee[:, a:b])
                    nc.sync.dma_start(out=out_d[q][:, a:b], in_=out2[:, a:b])


_NC_CACHE = {}


def _build_nc():
    if "nc" in _NC_CACHE:
        return _NC_CACHE["nc"]
    nc = bacc.Bacc()
    hxt_d = nc.dram_tensor("hxt", [NP, P, 2 * G * P], HXT_DT,
                           kind="ExternalInput")
    h_d = nc.dram_tensor("h2", [NP, P, 2 * HID], BF16, kind="ExternalInput")
    wt_d = nc.dram_tensor("wt", [P, GATE], WT_DT, kind="ExternalInput")
    out_d = nc.dram_tensor("out", [NP, P, 2 * HID], BF16,
                           kind="ExternalOutput")
    with tile.TileContext(nc) as tc:
        _body(tc, nc, hxt_d, h_d, wt_d, out_d)
    nc.compile()
    _NC_CACHE["nc"] = nc
    return nc


def _np_reference(x, h, weight, bias):
    hx = np.concatenate([h, x], axis=-1)
    xg = hx.reshape(x.shape[0], G, IN_PER)
    gate = np.einsum("bgi,goi->bgo", xg, weight).reshape(x.shape[0], GATE)
    gate = gate + bias
    r, c, u = np.split(gate, 3, axis=-1)
    reset = 1.0 / (1.0 + np.exp(-r))
    cand = np.tanh(reset * c)
    upd = 1.0 / (1.0 + np.exp(-u))
    return (upd * cand + (1.0 - upd) * h).astype(np.float32)


def _np_dt(dt):
    return {BF16: ml_dtypes.bfloat16,
            mybir.dt.float8e4: ml_dtypes.float8_e4m3}[dt]


def _pack_hxt(hs, xs):
    """-> [NP, 128, 4096] with hxt[q, p, 2048c+1024s+128g+b] =
    hx[256q+128s+b, 1024c+128g+p]: low half holds the transposed h blocks
    (pair-interleaved), high half the x blocks."""
    def tp(a):                      # [BS, 1024] -> [NP, 128, 1, 2048]
        return (a.reshape(NP, 2, P, 8, P)          # [q, s, b, g, p]
                .transpose(0, 4, 1, 3, 2)          # [q, p, s, g, b]
                .reshape(NP, P, 1, 2 * HID))
    arr = np.concatenate([tp(hs), tp(xs)], axis=2)  # [q, p, 2, 2048]
    return np.ascontiguousarray(arr.reshape(NP, P, 4 * HID)).astype(
        _np_dt(HXT_DT))


def _pack_pairs(a):
    """[BS, 1024] -> [NP, 128, 2048] with [q, p, 1024s+f] = a[256q+128s+p, f]."""
    return np.ascontiguousarray(
        a.reshape(NP, 2, P, HID).transpose(0, 2, 1, 3)
        .reshape(NP, P, 2 * HID))


def _unpack_pairs(a):
    """inverse of _pack_pairs."""
    return np.ascontiguousarray(
        a.reshape(NP, P, 2, HID).transpose(0, 2, 1, 3).reshape(BS, HID))


def _run(x, h, weight, bias, trace=False, tmpdir=None):
    # wt[p, 192g+o] = W[g, o, p] — the exact SBUF layout, one contiguous DMA
    wt = np.ascontiguousarray(
        (weight * WS).transpose(2, 0, 1).reshape(P, GATE)).astype(
        _np_dt(WT_DT))
    nc = _build_nc()
    in_maps = []
    for c in range(N_CORES):
        sl = slice(c * BS, (c + 1) * BS)
        xs, hs = x[sl], h[sl]
        in_maps.append({
            "hxt": _pack_hxt(hs, xs),
            "h2": _pack_pairs(hs).astype(ml_dtypes.bfloat16),
            "wt": wt,
        })
    res = run_bass_kernel_spmd(nc, in_maps, core_ids=list(range(N_CORES)),
                               trace=trace, tmpdir=tmpdir)
    out = np.concatenate(
        [_unpack_pairs(m["out"].astype(np.float32)) for m in res.results],
        axis=0)
    return out, res


def kernel(x, h, weight, bias):
    x = np.asarray(x, dtype=np.float32)
    h = np.asarray(h, dtype=np.float32)
    weight = np.asarray(weight, dtype=np.float32)
    bias = np.asarray(bias, dtype=np.float32)
    if np.any(bias != 0.0):
        # setup_inputs() always passes zero bias; keep a correct fallback.
        return _np_reference(x, h, weight, bias)
    out, _ = _run(x, h, weight, bias)
    return out


# revision 29
# speedup vs baseline: 1.2141x; 1.2141x over previous
"""BlockGRUCell Trainium2 kernel.

Computation (per reference):
  hx = concat([h, x], -1)                       # (B, 2048)
  gate[b, 192g+o] = sum_i hx[b, 128g+i] * W[g, o, i]   # block-diagonal matmul
  r, c, u = split(gate + bias, 3)               # bias == 0 from setup_inputs
  h_new = sigmoid(u) * tanh(sigmoid(r) * c) + (1 - sigmoid(u)) * h

Sharding: data-parallel over batch across 8 NeuronCores (2048 rows each),
weights replicated.

Design (evolved from the 95.7us baseline via trace analysis; ~70us now):
  - The baseline was DVE-bound (fp32 tensor_tensor is 1x ~= 1152ns per
    1024-wide op, 78us busy) with ACT (55us) and DMA (24MB -> 72us) close
    behind.  Fixes:
  - All elementwise tensors are bf16 -> DVE tensor_tensor runs 2x_1P
    (2 elem/cycle/lane).  Only the rc = sigmoid(r)*c multiply reads PSUM
    (f32, 1x) since TRN2 matmul can only write f32 PSUM.
  - h is loaded as bf16 (not f32) and the output is stored as bf16 and
    upcast on host: HBM traffic 24MB -> 16MB per core.
  - Tiles processed in pairs of 128 rows; tanh + blend run at free-dim
    2048 to amortize per-op overhead and semaphores.
  - ACTIVATEs are emitted with immediate bias (bass's float->AP bias
    conversion costs a measured ~92ns/op in const-AP reads).
  - Output stores go through the SWDGE (gpsimd) DMA ring so they never
    block the HWDGE (sync) load ring's FIFO dispatch; weight loads ride
    the second HWDGE ring (ACT-dispatched) in parallel with hxt loads.
  - Pair 0's panels arrive as 256KB pieces (sub-tile deps) so the first
    tile's matmuls start as early as possible during the pipeline fill.

Steady state is ACT/DVE co-bound at ~48-49us busy each (the 3 LUT
passes/element are irreducible: ACT is the only LUT engine at 1 elem/
cycle/lane; GpSimd has no PSUM access and its stock tensor ops deadlock
against DVE's shared-port lock, measured as NRT_EXEC_UNIT_UNRECOVERABLE).

Per core, per 128-row tile: 20 block matmuls (bf16, f32 PSUM, split at
512-col PSUM bank crossings) into gR/gC/gU [128,1024] panels; ACT does
sigmoid(gR)->reset, sigmoid(gU)->upd (bf16, into pair tiles); DVE does
rc = gC*reset (1x, PSUM).  Per pair: ACT tanh(rc_pair)->cand (2048 wide);
DVE: d = cand - h, e = upd*d, hn = h + e; store.
"""

import numpy as np
import ml_dtypes

import concourse.bass as bass
import concourse.bacc as bacc
import concourse.tile as tile
import concourse.mybir as mybir
from concourse.bass_utils import run_bass_kernel_spmd

N_CORES = 8
BATCH = 16384
BS = BATCH // N_CORES            # rows per core
P = 128
NT = BS // P                     # 128-row tiles per core
NP = NT // 2                     # tile pairs per core
HID = 1024
G = 16                           # feature blocks
IN_PER = 128
OUT_PER = 192
GATE = 3 * HID                   # 3072
PSUM_BANK_F32 = 512

F32 = mybir.dt.float32
BF16 = mybir.dt.bfloat16
AFT = mybir.ActivationFunctionType
MULT = mybir.AluOpType.mult

# dtype knobs. fp8e4 hxt was tried and reverted: the compiler never
# engaged fast-weight-load (LDWEIGHTS stayed ~128ns), the DMA savings were
# eaten by small-transfer inefficiency, and ACT/DVE op durations regressed
# ~1.2x in that build; bf16 also keeps 2x the error margin.
HXT_DT = BF16
WT_DT = BF16
WS = 1.0                         # weight pre-scale, unwound via ACT scale


def _act(nc, out, in_, func, scale=1.0):
    """ACTIVATE with immediate bias/scale/alpha.  bass's activation()
    lowers a float bias to a [P,1] const AP, whose per-op read costs
    ~92ns (HW-measured on 200-op chains); the ISA accepts immediates."""
    eng = nc.scalar
    ins = [eng.lower_ap(in_)]
    for val in (0.0, scale, 0.0):   # bias, scale, alpha
        ins.append(mybir.ImmediateValue(dtype=mybir.dt.float32, value=val))
    return eng.add_instruction(
        mybir.InstActivation(
            name=nc.get_next_instruction_name(),
            func=func,
            ins=ins,
            outs=[eng.lower_ap(out)],
        ))


def _body(tc, nc, hxt_d, h_d, wt_d, out_d):
    inv_ws = 1.0 / WS
    with (
        tc.tile_pool(name="consts", bufs=1) as consts,
        tc.tile_pool(name="io", bufs=3) as io,
        tc.tile_pool(name="panels", bufs=2) as panels,
        tc.tile_pool(name="gatep", bufs=4, space="PSUM") as gatep,
    ):
        # warm the sigmoid/tanh ACT table during the initial DMAs (the
        # ~2.7us ACT_TABLE_LOAD otherwise lands on tile 0's critical path)
        warm = consts.tile([P, 1], F32)
        nc.vector.memset(warm, 0.0)
        nc.scalar.activation(warm, warm, AFT.Sigmoid)

        # weight loads ride the second HWDGE ring (ACT engine dispatch, idle
        # at startup) so they stream in parallel with the hxt loads on the
        # sync ring, in r/c/u pieces so each gate region unblocks asap
        wt_s = consts.tile([P, GATE], WT_DT)
        for k in range(3):
            nc.scalar.dma_start(out=wt_s[:, k * HID:(k + 1) * HID],
                                in_=wt_d[:, k * HID:(k + 1) * HID])

        for q in range(NP):
            # one [128, 4096] tile per pair, h-blocks in the low half,
            # x-blocks in the high half; steady state is one contiguous 1MB
            # load, pair 0 arrives in three pieces (sub-tile deps let tile
            # 0's h-block matmuls start after the first 256KB)
            hxt = io.tile([P, 2 * G * P], HXT_DT, tag="hxt")
            if q == 0:
                nc.sync.dma_start(out=hxt[:, 0:HID], in_=hxt_d[0, :, 0:HID])
                nc.sync.dma_start(out=hxt[:, HID:2 * HID],
                                  in_=hxt_d[0, :, HID:2 * HID])
                nc.sync.dma_start(out=hxt[:, 2 * HID:],
                                  in_=hxt_d[0, :, 2 * HID:])
            else:
                nc.sync.dma_start(out=hxt, in_=hxt_d[q])
            h2 = io.tile([P, 2 * HID], BF16, tag="h2")
            if q > 0:
                nc.sync.dma_start(out=h2, in_=h_d[q])
            out2 = io.tile([P, 2 * HID], BF16, tag="out2")

            reset2 = panels.tile([P, 2 * HID], BF16, tag="reset")
            upd2 = panels.tile([P, 2 * HID], BF16, tag="upd")
            rc2 = panels.tile([P, 2 * HID], BF16, tag="rc")
            cand2 = panels.tile([P, 2 * HID], BF16, tag="cand")
            dd = panels.tile([P, 2 * HID], BF16, tag="dd")
            ee = panels.tile([P, 2 * HID], BF16, tag="ee")

            for s in (0, 1):
                gR = gatep.tile([P, HID], F32, tag="gate")
                gC = gatep.tile([P, HID], F32, tag="gate")
                gU = gatep.tile([P, HID], F32, tag="gate")
                gs = (gR, gC, gU)

                for g in range(G):
                    base = 2 * HID * (g // 8) + HID * s + (g % 8) * P
                    lhsT = hxt[:, base:base + P]
                    w0 = g * OUT_PER
                    # split matmul writes at PSUM bank (512 f32) boundaries
                    c0 = w0
                    while c0 < w0 + OUT_PER:
                        c1 = min(w0 + OUT_PER,
                                 (c0 // PSUM_BANK_F32 + 1) * PSUM_BANK_F32)
                        gate = gs[c0 // HID]
                        nc.tensor.matmul(
                            gate[:, c0 % HID:(c0 % HID) + c1 - c0],
                            lhsT, wt_s[:, c0:c1], start=True, stop=True)
                        c0 = c1

                if q == 0 and s == 0:
                    # defer the first h load until after the critical-path
                    # hxt/wt DMAs are queued (h is only needed at the blend)
                    nc.sync.dma_start(out=h2, in_=h_d[0])

                sl = slice(s * HID, (s + 1) * HID)
                _act(nc, reset2[:, sl], gR, AFT.Sigmoid, inv_ws)
                _act(nc, upd2[:, sl], gU, AFT.Sigmoid, inv_ws)
                if q == NP - 1 and s == 1:
                    # the very last rc is on the drain-tail critical path:
                    # run it in halves so the final tanh piece starts sooner
                    h0, hm = s * HID, s * HID + HID // 2
                    nc.vector.tensor_tensor(rc2[:, h0:hm], gC[:, 0:HID // 2],
                                            reset2[:, h0:hm], MULT)
                    nc.vector.tensor_tensor(rc2[:, hm:hm + HID // 2],
                                            gC[:, HID // 2:],
                                            reset2[:, hm:hm + HID // 2], MULT)
                else:
                    nc.vector.tensor_tensor(rc2[:, sl], gC, reset2[:, sl],
                                            MULT)

            # pair epilogue at free-dim 2048; the last pair's DVE chain runs
            # in 512-wide quarters (tanh in halves) so the serial ACT<->DVE
            # tail drains finer-grained and the final store streams out early
            if q < NP - 1:
                tanh_splits = [(0, 2 * HID)]
                dve_splits = [(0, 2 * HID)]
            else:
                tanh_splits = [(0, HID), (HID, HID + HID // 2),
                               (HID + HID // 2, 2 * HID)]
                dve_splits = tanh_splits
            for a, b in tanh_splits:
                _act(nc, cand2[:, a:b], rc2[:, a:b], AFT.Tanh, inv_ws)
            for a, b in dve_splits:
                nc.vector.tensor_sub(dd[:, a:b], cand2[:, a:b], h2[:, a:b])
                nc.vector.tensor_mul(ee[:, a:b], upd2[:, a:b], dd[:, a:b])
                nc.vector.tensor_add(out2[:, a:b], h2[:, a:b], ee[:, a:b])
                # stores ride the SWDGE (gpsimd) ring: the HWDGE load ring
                # dispatches FIFO, so a store waiting on compute would stall
                # the next pair's loads.  The last two pairs' stores go on
                # the (by then idle) sync ring instead — SWDGE descriptors
                # starve behind DVE port locks and its final drain is slow.
                if q < NP - 2:
                    nc.gpsimd.dma_start(out=out_d[q][:, a:b], in_=out2[:, a:b])
                else:
                    nc.sync.dma_start(out=out_d[q][:, a:b], in_=out2[:, a:b])


_NC_CACHE = {}


def _build_nc():
    if "nc" in _NC_CACHE:
        return _NC_CACHE["nc"]
    nc = bacc.Bacc()
    hxt_d = nc.dram_tensor("hxt", [NP, P, 2 * G * P], HXT_DT,
                           kind="ExternalInput")
    h_d = nc.dram_tensor("h2", [NP, P, 2 * HID], BF16, kind="ExternalInput")
    wt_d = nc.dram_tensor("wt", [P, GATE], WT_DT, kind="ExternalInput")
    out_d = nc.dram_tensor("out", [NP, P, 2 * HID], BF16,
                           kind="ExternalOutput")
    with tile.TileContext(nc) as tc:
        _body(tc, nc, hxt_d, h_d, wt_d, out_d)
    nc.compile()
    _NC_CACHE["nc"] = nc
    return nc


def _np_reference(x, h, weight, bias):
    hx = np.concatenate([h, x], axis=-1)
    xg = hx.reshape(x.shape[0], G, IN_PER)
    gate = np.einsum("bgi,goi->bgo", xg, weight).reshape(x.shape[0], GATE)
    gate = gate + bias
    r, c, u = np.split(gate, 3, axis=-1)
    reset = 1.0 / (1.0 + np.exp(-r))
    cand = np.tanh(reset * c)
    upd = 1.0 / (1.0 + np.exp(-u))
    return (upd * cand + (1.0 - upd) * h).astype(np.float32)


def _np_dt(dt):
    return {BF16: ml_dtypes.bfloat16,
            mybir.dt.float8e4: ml_dtypes.float8_e4m3}[dt]


def _pack_hxt(hs, xs):
    """-> [NP, 128, 4096] with hxt[q, p, 2048c+1024s+128g+b] =
    hx[256q+128s+b, 1024c+128g+p]: low half holds the transposed h blocks
    (pair-interleaved), high half the x blocks."""
    def tp(a):                      # [BS, 1024] -> [NP, 128, 1, 2048]
        return (a.reshape(NP, 2, P, 8, P)          # [q, s, b, g, p]
                .transpose(0, 4, 1, 3, 2)          # [q, p, s, g, b]
                .reshape(NP, P, 1, 2 * HID))
    arr = np.concatenate([tp(hs), tp(xs)], axis=2)  # [q, p, 2, 2048]
    return np.ascontiguousarray(arr.reshape(NP, P, 4 * HID)).astype(
        _np_dt(HXT_DT))


def _pack_pairs(a):
    """[BS, 1024] -> [NP, 128, 2048] with [q, p, 1024s+f] = a[256q+128s+p, f]."""
    return np.ascontiguousarray(
        a.reshape(NP, 2, P, HID).transpose(0, 2, 1, 3)
        .reshape(NP, P, 2 * HID))


def _unpack_pairs(a):
    """inverse of _pack_pairs."""
    return np.ascontiguousarray(
        a.reshape(NP, P, 2, HID).transpose(0, 2, 1, 3).reshape(BS, HID))


def _run(x, h, weight, bias, trace=False, tmpdir=None):
    # wt[p, 192g+o] = W[g, o, p] — the exact SBUF layout, one contiguous DMA
    wt = np.ascontiguousarray(
        (weight * WS).transpose(2, 0, 1).reshape(P, GATE)).astype(
        _np_dt(WT_DT))
    nc = _build_nc()
    in_maps = []
    for c in range(N_CORES):
        sl = slice(c * BS, (c + 1) * BS)
        xs, hs = x[sl], h[sl]
        in_maps.append({
            "hxt": _pack_hxt(hs, xs),
            "h2": _pack_pairs(hs).astype(ml_dtypes.bfloat16),
            "wt": wt,
        })
    res = run_bass_kernel_spmd(nc, in_maps, core_ids=list(range(N_CORES)),
                               trace=trace, tmpdir=tmpdir)
    out = np.concatenate(
        [_unpack_pairs(m["out"].astype(np.float32)) for m in res.results],
        axis=0)
    return out, res


def kernel(x, h, weight, bias):
    x = np.asarray(x, dtype=np.float32)
    h = np.asarray(h, dtype=np.float32)
    weight = np.asarray(weight, dtype=np.float32)
    bias = np.asarray(bias, dtype=np.float32)
    if np.any(bias != 0.0):
        # setup_inputs() always passes zero bias; keep a correct fallback.
        return _np_reference(x, h, weight, bias)
    out, _ = _run(x, h, weight, bias)
    return out


# revision 30
# speedup vs baseline: 1.2367x; 1.0187x over previous
"""BlockGRUCell Trainium2 kernel.

Computation (per reference):
  hx = concat([h, x], -1)                       # (B, 2048)
  gate[b, 192g+o] = sum_i hx[b, 128g+i] * W[g, o, i]   # block-diagonal matmul
  r, c, u = split(gate + bias, 3)               # bias == 0 from setup_inputs
  h_new = sigmoid(u) * tanh(sigmoid(r) * c) + (1 - sigmoid(u)) * h

Sharding: data-parallel over batch across 8 NeuronCores (2048 rows each),
weights replicated.

Design (evolved from the 95.7us baseline via trace analysis; ~70us now):
  - The baseline was DVE-bound (fp32 tensor_tensor is 1x ~= 1152ns per
    1024-wide op, 78us busy) with ACT (55us) and DMA (24MB -> 72us) close
    behind.  Fixes:
  - All elementwise tensors are bf16 -> DVE tensor_tensor runs 2x_1P
    (2 elem/cycle/lane).  Only the rc = sigmoid(r)*c multiply reads PSUM
    (f32, 1x) since TRN2 matmul can only write f32 PSUM.
  - h is loaded as bf16 (not f32) and the output is stored as bf16 and
    upcast on host: HBM traffic 24MB -> 16MB per core.
  - Tiles processed in pairs of 128 rows; tanh + blend run at free-dim
    2048 to amortize per-op overhead and semaphores.
  - ACTIVATEs are emitted with immediate bias (bass's float->AP bias
    conversion costs a measured ~92ns/op in const-AP reads).
  - Output stores go through the SWDGE (gpsimd) DMA ring so they never
    block the HWDGE (sync) load ring's FIFO dispatch; weight loads ride
    the second HWDGE ring (ACT-dispatched) in parallel with hxt loads.
  - Pair 0's panels arrive as 256KB pieces (sub-tile deps) so the first
    tile's matmuls start as early as possible during the pipeline fill.

Steady state is ACT/DVE co-bound at ~48-49us busy each (the 3 LUT
passes/element are irreducible: ACT is the only LUT engine at 1 elem/
cycle/lane; GpSimd has no PSUM access and its stock tensor ops deadlock
against DVE's shared-port lock, measured as NRT_EXEC_UNIT_UNRECOVERABLE).

Per core, per 128-row tile: 20 block matmuls (bf16, f32 PSUM, split at
512-col PSUM bank crossings) into gR/gC/gU [128,1024] panels; ACT does
sigmoid(gR)->reset, sigmoid(gU)->upd (bf16, into pair tiles); DVE does
rc = gC*reset (1x, PSUM).  Per pair: ACT tanh(rc_pair)->cand (2048 wide);
DVE: d = cand - h, e = upd*d, hn = h + e; store.
"""

import numpy as np
import ml_dtypes

import concourse.bass as bass
import concourse.bacc as bacc
import concourse.tile as tile
import concourse.mybir as mybir
from concourse.bass_utils import run_bass_kernel_spmd

N_CORES = 8
BATCH = 16384
BS = BATCH // N_CORES            # rows per core
P = 128
NT = BS // P                     # 128-row tiles per core
NP = NT // 2                     # tile pairs per core
HID = 1024
G = 16                           # feature blocks
IN_PER = 128
OUT_PER = 192
GATE = 3 * HID                   # 3072
PSUM_BANK_F32 = 512

F32 = mybir.dt.float32
BF16 = mybir.dt.bfloat16
AFT = mybir.ActivationFunctionType
MULT = mybir.AluOpType.mult

# dtype knobs. fp8e4 hxt was tried and reverted: the compiler never
# engaged fast-weight-load (LDWEIGHTS stayed ~128ns), the DMA savings were
# eaten by small-transfer inefficiency, and ACT/DVE op durations regressed
# ~1.2x in that build; bf16 also keeps 2x the error margin.
HXT_DT = BF16
WT_DT = BF16
WS = 1.0                         # weight pre-scale, unwound via ACT scale


def _act(nc, out, in_, func, scale=1.0):
    """ACTIVATE with immediate bias/scale/alpha.  bass's activation()
    lowers a float bias to a [P,1] const AP, whose per-op read costs
    ~92ns (HW-measured on 200-op chains); the ISA accepts immediates."""
    eng = nc.scalar
    ins = [eng.lower_ap(in_)]
    for val in (0.0, scale, 0.0):   # bias, scale, alpha
        ins.append(mybir.ImmediateValue(dtype=mybir.dt.float32, value=val))
    return eng.add_instruction(
        mybir.InstActivation(
            name=nc.get_next_instruction_name(),
            func=func,
            ins=ins,
            outs=[eng.lower_ap(out)],
        ))


def _body(tc, nc, hxt_d, h_d, wt_d, out_d):
    inv_ws = 1.0 / WS
    with (
        tc.tile_pool(name="consts", bufs=1) as consts,
        tc.tile_pool(name="io", bufs=3) as io,
        tc.tile_pool(name="panels", bufs=2) as panels,
        tc.tile_pool(name="gatep", bufs=4, space="PSUM") as gatep,
    ):
        # warm the sigmoid/tanh ACT table during the initial DMAs (the
        # ~2.7us ACT_TABLE_LOAD otherwise lands on tile 0's critical path)
        warm = consts.tile([P, 1], F32)
        nc.vector.memset(warm, 0.0)
        nc.scalar.activation(warm, warm, AFT.Sigmoid)

        # weight loads ride the second HWDGE ring (ACT engine dispatch, idle
        # at startup) so they stream in parallel with the hxt loads on the
        # sync ring, in r/c/u pieces so each gate region unblocks asap
        wt_s = consts.tile([P, GATE], WT_DT)
        for k in range(3):
            nc.scalar.dma_start(out=wt_s[:, k * HID:(k + 1) * HID],
                                in_=wt_d[:, k * HID:(k + 1) * HID])

        for q in range(NP):
            # one [128, 4096] tile per pair, h-blocks in the low half,
            # x-blocks in the high half; steady state is one contiguous 1MB
            # load, pair 0 arrives in three pieces (sub-tile deps let tile
            # 0's h-block matmuls start after the first 256KB)
            hxt = io.tile([P, 2 * G * P], HXT_DT, tag="hxt")
            if q == 0:
                nc.sync.dma_start(out=hxt[:, 0:HID], in_=hxt_d[0, :, 0:HID])
                nc.sync.dma_start(out=hxt[:, HID:2 * HID],
                                  in_=hxt_d[0, :, HID:2 * HID])
                nc.sync.dma_start(out=hxt[:, 2 * HID:],
                                  in_=hxt_d[0, :, 2 * HID:])
            else:
                nc.sync.dma_start(out=hxt, in_=hxt_d[q])
            h2 = io.tile([P, 2 * HID], BF16, tag="h2")
            if q > 0:
                nc.sync.dma_start(out=h2, in_=h_d[q])
            out2 = io.tile([P, 2 * HID], BF16, tag="out2")

            reset2 = panels.tile([P, 2 * HID], BF16, tag="reset")
            upd2 = panels.tile([P, 2 * HID], BF16, tag="upd")
            rc2 = panels.tile([P, 2 * HID], BF16, tag="rc")
            cand2 = panels.tile([P, 2 * HID], BF16, tag="cand")
            dd = panels.tile([P, 2 * HID], BF16, tag="dd")
            ee = panels.tile([P, 2 * HID], BF16, tag="ee")

            for s in (0, 1):
                gR = gatep.tile([P, HID], F32, tag="gate")
                gC = gatep.tile([P, HID], F32, tag="gate")
                gU = gatep.tile([P, HID], F32, tag="gate")
                gs = (gR, gC, gU)

                for g in range(G):
                    base = 2 * HID * (g // 8) + HID * s + (g % 8) * P
                    lhsT = hxt[:, base:base + P]
                    w0 = g * OUT_PER
                    # split matmul writes at PSUM bank (512 f32) boundaries
                    c0 = w0
                    while c0 < w0 + OUT_PER:
                        c1 = min(w0 + OUT_PER,
                                 (c0 // PSUM_BANK_F32 + 1) * PSUM_BANK_F32)
                        gate = gs[c0 // HID]
                        nc.tensor.matmul(
                            gate[:, c0 % HID:(c0 % HID) + c1 - c0],
                            lhsT, wt_s[:, c0:c1], start=True, stop=True)
                        c0 = c1

                if q == 0 and s == 0:
                    # defer the first h load until after the critical-path
                    # hxt/wt DMAs are queued (h is only needed at the blend)
                    nc.sync.dma_start(out=h2, in_=h_d[0])

                sl = slice(s * HID, (s + 1) * HID)
                _act(nc, reset2[:, sl], gR, AFT.Sigmoid, inv_ws)
                _act(nc, upd2[:, sl], gU, AFT.Sigmoid, inv_ws)
                nc.vector.tensor_tensor(rc2[:, sl], gC, reset2[:, sl], MULT)

            # pair epilogue at free-dim 2048; the last pair's DVE chain runs
            # in 512-wide quarters (tanh in halves) so the serial ACT<->DVE
            # tail drains finer-grained and the final store streams out early
            if q < NP - 1:
                tanh_splits = [(0, 2 * HID)]
                dve_splits = [(0, 2 * HID)]
            else:
                tanh_splits = [(0, HID), (HID, 2 * HID)]
                dve_splits = [(0, HID), (HID, 2 * HID)]
            for a, b in tanh_splits:
                _act(nc, cand2[:, a:b], rc2[:, a:b], AFT.Tanh, inv_ws)
            for a, b in dve_splits:
                nc.vector.tensor_sub(dd[:, a:b], cand2[:, a:b], h2[:, a:b])
                nc.vector.tensor_mul(ee[:, a:b], upd2[:, a:b], dd[:, a:b])
                nc.vector.tensor_add(out2[:, a:b], h2[:, a:b], ee[:, a:b])
                # stores ride the SWDGE (gpsimd) ring: the HWDGE load ring
                # dispatches FIFO, so a store waiting on compute would stall
                # the next pair's loads.  The last two pairs' stores go on
                # the (by then idle) sync ring instead — SWDGE descriptors
                # starve behind DVE port locks and its final drain is slow.
                if q < NP - 2:
                    nc.gpsimd.dma_start(out=out_d[q][:, a:b], in_=out2[:, a:b])
                else:
                    nc.sync.dma_start(out=out_d[q][:, a:b], in_=out2[:, a:b])


_NC_CACHE = {}


def _build_nc():
    if "nc" in _NC_CACHE:
        return _NC_CACHE["nc"]
    nc = bacc.Bacc()
    hxt_d = nc.dram_tensor("hxt", [NP, P, 2 * G * P], HXT_DT,
                           kind="ExternalInput")
    h_d = nc.dram_tensor("h2", [NP, P, 2 * HID], BF16, kind="ExternalInput")
    wt_d = nc.dram_tensor("wt", [P, GATE], WT_DT, kind="ExternalInput")
    out_d = nc.dram_tensor("out", [NP, P, 2 * HID], BF16,
                           kind="ExternalOutput")
    with tile.TileContext(nc) as tc:
        _body(tc, nc, hxt_d, h_d, wt_d, out_d)
    nc.compile()
    _NC_CACHE["nc"] = nc
    return nc


def _np_reference(x, h, weight, bias):
    hx = np.concatenate([h, x], axis=-1)
    xg = hx.reshape(x.shape[0], G, IN_PER)
    gate = np.einsum("bgi,goi->bgo", xg, weight).reshape(x.shape[0], GATE)
    gate = gate + bias
    r, c, u = np.split(gate, 3, axis=-1)
    reset = 1.0 / (1.0 + np.exp(-r))
    cand = np.tanh(reset * c)
    upd = 1.0 / (1.0 + np.exp(-u))
    return (upd * cand + (1.0 - upd) * h).astype(np.float32)


def _np_dt(dt):
    return {BF16: ml_dtypes.bfloat16,
            mybir.dt.float8e4: ml_dtypes.float8_e4m3}[dt]


def _pack_hxt(hs, xs):
    """-> [NP, 128, 4096] with hxt[q, p, 2048c+1024s+128g+b] =
    hx[256q+128s+b, 1024c+128g+p]: low half holds the transposed h blocks
    (pair-interleaved), high half the x blocks."""
    def tp(a):                      # [BS, 1024] -> [NP, 128, 1, 2048]
        return (a.reshape(NP, 2, P, 8, P)          # [q, s, b, g, p]
                .transpose(0, 4, 1, 3, 2)          # [q, p, s, g, b]
                .reshape(NP, P, 1, 2 * HID))
    arr = np.concatenate([tp(hs), tp(xs)], axis=2)  # [q, p, 2, 2048]
    return np.ascontiguousarray(arr.reshape(NP, P, 4 * HID)).astype(
        _np_dt(HXT_DT))


def _pack_pairs(a):
    """[BS, 1024] -> [NP, 128, 2048] with [q, p, 1024s+f] = a[256q+128s+p, f]."""
    return np.ascontiguousarray(
        a.reshape(NP, 2, P, HID).transpose(0, 2, 1, 3)
        .reshape(NP, P, 2 * HID))


def _unpack_pairs(a):
    """inverse of _pack_pairs."""
    return np.ascontiguousarray(
        a.reshape(NP, P, 2, HID).transpose(0, 2, 1, 3).reshape(BS, HID))


def _run(x, h, weight, bias, trace=False, tmpdir=None):
    # wt[p, 192g+o] = W[g, o, p] — the exact SBUF layout, one contiguous DMA
    wt = np.ascontiguousarray(
        (weight * WS).transpose(2, 0, 1).reshape(P, GATE)).astype(
        _np_dt(WT_DT))
    nc = _build_nc()
    in_maps = []
    for c in range(N_CORES):
        sl = slice(c * BS, (c + 1) * BS)
        xs, hs = x[sl], h[sl]
        in_maps.append({
            "hxt": _pack_hxt(hs, xs),
            "h2": _pack_pairs(hs).astype(ml_dtypes.bfloat16),
            "wt": wt,
        })
    res = run_bass_kernel_spmd(nc, in_maps, core_ids=list(range(N_CORES)),
                               trace=trace, tmpdir=tmpdir)
    out = np.concatenate(
        [_unpack_pairs(m["out"].astype(np.float32)) for m in res.results],
        axis=0)
    return out, res


def kernel(x, h, weight, bias):
    x = np.asarray(x, dtype=np.float32)
    h = np.asarray(h, dtype=np.float32)
    weight = np.asarray(weight, dtype=np.float32)
    bias = np.asarray(bias, dtype=np.float32)
    if np.any(bias != 0.0):
        # setup_inputs() always passes zero bias; keep a correct fallback.
        return _np_reference(x, h, weight, bias)
    out, _ = _run(x, h, weight, bias)
    return out
